# revision 1
# baseline (speedup 1.0000x reference)
"""Trainium2 Bass kernel for 2D-relative-bias multi-head attention.

Shapes (hardcoded): x [64, 16, 16, 512], 16 heads x 32 dim, S = 256.
Sharding: data-parallel over batch, 8 batches per core on 8 cores.

Per-core device pipeline (all matmuls bf16, fp32 PSUM accumulation):
  qT/kT = W^T @ x^T            [nd, tok]   (PE, K=c)
  v     = x @ Wv               [tok, nd]   (PE)
  logitsT[t,s] 4 heads/tile    (PE, K=32, 4-way row packing -> concurrent)
  E0 = exp(logitsT)            (ACT, PSUM->SBUF bf16, [128,1024] instrs)
  E  = E0 * exp(biasT)         (DVE + Pool split, bias table from host)
  sums = 1^T E (replicated)    (PE, 4-head col-packed, all-ones lhsT)
  out_unT = V^T E              (PE, 4-head col-packed)
  R = 1/sums                   (DVE reciprocal_approx_fast)
  outT = out_unT * R           (DVE)
  final = outT^T @ Wo + o_b    (PE) -> bf16 -> DMA (host casts to f32)
"""

import numpy as np
import ml_dtypes

try:
    import concourse.bass as bass
except ImportError:  # pragma: no cover
    import sys

    sys.path.insert(0, "/opt/trn_rl_repo")
    import concourse.bass as bass
from concourse import bacc

import concourse.mybir as mybir
import concourse.tile as tile
from concourse.bass_utils import run_bass_kernel_spmd

BF16 = mybir.dt.bfloat16
F32 = mybir.dt.float32
FP8 = mybir.dt.float8e4
FP8_WSCALE = 64.0
AF = mybir.ActivationFunctionType
OP = mybir.AluOpType

B, H, W, C = 64, 16, 16, 512
NH, D = 16, 32
S = H * W            # 256
NCORES = 8
BPC = B // NCORES    # 8 batches per core
TOK = BPC * S        # 2048 tokens per core
SCALE = D ** -0.5

def build_program(reps: int = 1, debug: bool = False,
                  sections=('qkv', 'attn', 'sums', 'av', 'out'),
                  with_qkbias: bool = False, use_fp8: bool = True,
                  bias_mode: str = 'mult', pool_heads: int = 8,
                  pa_in_pl: bool = True, v_act: bool = False,
                  qk_bufs: int = 1, big_ps: bool = True):
    nc = bacc.Bacc()
    xT_d = nc.dram_tensor("xT", [128, 4 * TOK], BF16, kind="ExternalInput")
    if use_fp8:
        x8_d = nc.dram_tensor("x8", [128, 4 * TOK], FP8, kind="ExternalInput")
        w8_d = nc.dram_tensor("w8", [128, 8 * 512], FP8, kind="ExternalInput")
        wqkvo_d = nc.dram_tensor("wvo_t", [128, 8 * 512], BF16, kind="ExternalInput")
    else:
        wqkvo_d = nc.dram_tensor("wqkvo", [128, 16 * 512], BF16, kind="ExternalInput")
    if bias_mode == 'pe':
        biasT_d = nc.dram_tensor("biasT", [128, 2 * NH * S], BF16, kind="ExternalInput")
        ident_d = nc.dram_tensor("ident", [128, 32], BF16, kind="ExternalInput")
    else:
        expb_d = nc.dram_tensor("expb", [128, 2 * NH * S], BF16, kind="ExternalInput")
    qb_d = nc.dram_tensor("qb", [1, 512], BF16, kind="ExternalInput")
    kb_d = nc.dram_tensor("kb", [1, 512], BF16, kind="ExternalInput")
    ones_r_d = nc.dram_tensor("ones_r", [1, 512], BF16, kind="ExternalInput")
    ones_c_d = nc.dram_tensor("ones_c", [128, 32], BF16, kind="ExternalInput")
    out_d = nc.dram_tensor("out", [TOK, 512], BF16, kind="ExternalOutput")

    with tile.TileContext(nc) as tc:
        import contextlib

        with contextlib.ExitStack() as ctx:
            wpool = ctx.enter_context(tc.tile_pool(name="wpool", bufs=1))
            xpool = ctx.enter_context(tc.tile_pool(name="xpool", bufs=1))
            qkpool = ctx.enter_context(tc.tile_pool(name="qkpool", bufs=1))
            epool = ctx.enter_context(tc.tile_pool(name="epool", bufs=3))
            rpool = ctx.enter_context(tc.tile_pool(
                name="rpool", bufs=1 if qk_bufs > 1 else 2))
            otpool = ctx.enter_context(tc.tile_pool(
                name="otpool", bufs=2 if qk_bufs > 1 else 4))
            fpool = ctx.enter_context(tc.tile_pool(
                name="fpool", bufs=1 if qk_bufs > 1 else 2))
            pl_pool = ctx.enter_context(
                tc.tile_pool(name="pl",
                             bufs=2 if big_ps else (3 if pa_in_pl else 2),
                             space="PSUM"))
            pa_pool = (pl_pool if pa_in_pl else ctx.enter_context(
                tc.tile_pool(name="pa", bufs=1, space="PSUM")))
            ps_pool = ctx.enter_context(
                tc.tile_pool(name="ps", bufs=2, space="PSUM"))

            # ---- persistent constants ----
            if use_fp8:
                wall = wpool.tile([128, 8 * 512], BF16, name="wall", tag="wall")
                nc.sync.dma_start(wall[:], wqkvo_d[:])
                wv = [wall[:, i * 512:(i + 1) * 512] for i in range(4)]
                wo = [wall[:, (4 + i) * 512:(5 + i) * 512] for i in range(4)]
                x8_all = wpool.tile([128, 4 * TOK], FP8, name="x8", tag="x8")
                nc.sync.dma_start(x8_all[:], x8_d[:])
                x8 = x8_all.rearrange("p (k t) -> p k t", k=4)
                w8_all = wpool.tile([128, 8 * 512], FP8, name="w8", tag="w8")
                nc.sync.dma_start(w8_all[:], w8_d[:])
                w8 = [w8_all[:, j * 2048:(j + 1) * 2048].rearrange(
                    "p (k n) -> p k n", k=4) for j in range(2)]
            else:
                wall = wpool.tile([128, 16 * 512], BF16, name="wall", tag="wall")
                nc.sync.dma_start(wall[:], wqkvo_d[:])
                wq = [wall[:, (3 * i + 0) * 512:(3 * i + 1) * 512] for i in range(4)]
                wk = [wall[:, (3 * i + 1) * 512:(3 * i + 2) * 512] for i in range(4)]
                wv = [wall[:, (3 * i + 2) * 512:(3 * i + 3) * 512] for i in range(4)]
                wo = [wall[:, (12 + i) * 512:(13 + i) * 512] for i in range(4)]
            if bias_mode == 'pe':
                biasT_all = wpool.tile([128, 2 * NH * S], BF16, name="biasT", tag="biasT")
                nc.sync.dma_start(biasT_all[:], biasT_d[:])
                biasT_v = biasT_all.rearrange("p (t n s) -> p t n s", t=2, n=NH)
                ident = wpool.tile([128, 32], BF16, name="ident", tag="ident")
                nc.sync.dma_start(ident[:], ident_d[:])
            else:
                expb_all = wpool.tile([128, 2 * NH * S], BF16, name="expb", tag="expb")
                nc.sync.dma_start(expb_all[:], expb_d[:])
                expb = [expb_all[:, t * NH * S:(t + 1) * NH * S] for t in range(2)]
            qb = wpool.tile([1, 512], BF16, name="qb", tag="qb")
            kb = wpool.tile([1, 512], BF16, name="kb", tag="kb")
            ones_r = wpool.tile([1, 512], BF16, name="ones_r", tag="ones_r")
            ones_c = wpool.tile([128, 32], BF16, name="ones_c", tag="ones_c")
            nc.sync.dma_start(qb[:], qb_d[:])
            nc.sync.dma_start(kb[:], kb_d[:])
            nc.sync.dma_start(ones_r[:], ones_r_d[:])
            nc.sync.dma_start(ones_c[:], ones_c_d[:])
            xT_all = xpool.tile([128, 4 * TOK], BF16, name="xT", tag="xT")
            nc.sync.dma_start(xT_all[:], xT_d[:])
            xT = [xT_all[:, i * TOK:(i + 1) * TOK] for i in range(4)]

            for _rep in range(reps):
                do = lambda s: s in sections
                # ---- phase 1: QKV projection chunk emitter (interleaved) ----
                if big_ps:
                    qTall = qkpool.tile([128, 4 * TOK], BF16, name="qTall",
                                        tag="qTall", bufs=1)
                    kTall = qkpool.tile([128, 4 * TOK], BF16, name="kTall",
                                        tag="kTall", bufs=1)
                    vall = qkpool.tile([128, 16 * 512], BF16, name="vall",
                                       tag="vall", bufs=1)
                    qT = [qTall[:, m * TOK:(m + 1) * TOK] for m in range(4)]
                    kT = [kTall[:, m * TOK:(m + 1) * TOK] for m in range(4)]
                    v_sb = [vall[:, s * 512:(s + 1) * 512]
                            for s in range(TOK // 128)]
                else:
                    qT = [qkpool.tile([128, TOK], BF16, name=f"qT{m}",
                                      tag=f"qT{m}", bufs=qk_bufs) for m in range(4)]
                    kT = [qkpool.tile([128, TOK], BF16, name=f"kT{m}",
                                      tag=f"kT{m}", bufs=qk_bufs) for m in range(4)]
                    v_sb = [qkpool.tile([128, 512], BF16, name=f"v{s}",
                                        tag=f"v{s}", bufs=1)
                            for s in range(TOK // 128)]

                DR = mybir.MatmulPerfMode.DoubleRow
                INV_WS = 1.0 / FP8_WSCALE

                def emit_qkv_chunk(nch):
                    """q,k projections for token chunk nch (512 tokens) + v for its 4 s-chunks."""
                    sl = slice(nch * 512, (nch + 1) * 512)
                    if use_fp8 and big_ps:
                        for j, dall in ((0, qTall), (1, kTall)):
                            for mp in range(2):
                                ps = ps_pool.tile([128, 1024], F32, name="ps", tag="ps")
                                for mi in range(2):
                                    m = 2 * mp + mi
                                    for kp in range(2):
                                        nc.tensor.matmul(
                                            ps[:, mi * 512:(mi + 1) * 512],
                                            w8[j][:, 2 * kp:2 * kp + 2,
                                                  m * 128:(m + 1) * 128],
                                            x8[:, 2 * kp:2 * kp + 2, sl],
                                            start=(kp == 0), stop=(kp == 1),
                                            perf_mode=DR)
                                dst = dall.rearrange(
                                    "p (m t) -> p m t", m=4
                                )[:, 2 * mp:2 * mp + 2, sl]
                                nc.vector.tensor_scalar_mul(
                                    dst, ps.rearrange("p (m t) -> p m t", m=2),
                                    INV_WS)
                        for sp in range(2):
                            ps = ps_pool.tile([128, 1024], F32, name="ps", tag="ps")
                            for si in range(2):
                                sch = nch * 4 + 2 * sp + si
                                for kc in range(4):
                                    nc.tensor.matmul(
                                        ps[:, si * 512:(si + 1) * 512],
                                        xT[kc][:, sch * 128:(sch + 1) * 128],
                                        wv[kc][:, :512],
                                        start=(kc == 0), stop=(kc == 3))
                            sch0 = nch * 4 + 2 * sp
                            nc.vector.tensor_copy(
                                vall[:, sch0 * 512:(sch0 + 2) * 512], ps[:])
                        return
                    if use_fp8:
                        for j, dst in ((0, qT), (1, kT)):
                            for m in range(4):
                                ps = ps_pool.tile([128, 512], F32, name="ps", tag="ps")
                                for kp in range(2):
                                    nc.tensor.matmul(
                                        ps[:, :512],
                                        w8[j][:, 2 * kp:2 * kp + 2,
                                              m * 128:(m + 1) * 128],
                                        x8[:, 2 * kp:2 * kp + 2, sl],
                                        start=(kp == 0), stop=(kp == 1),
                                        perf_mode=DR)
                                nc.vector.tensor_scalar_mul(
                                    dst[m][:, sl], ps[:, :512], INV_WS)
                        for sch in range(nch * 4, (nch + 1) * 4):
                            ps = ps_pool.tile([128, 512], F32, name="ps", tag="ps")
                            for kc in range(4):
                                nc.tensor.matmul(
                                    ps[:, :512],
                                    xT[kc][:, sch * 128:(sch + 1) * 128],
                                    wv[kc][:, :512],
                                    start=(kc == 0), stop=(kc == 3))
                            if v_act:
                                nc.scalar.activation(
                                    v_sb[sch][:], ps[:, :512], AF.Copy)
                            else:
                                nc.vector.tensor_copy(v_sb[sch][:], ps[:, :512])
                        return
                    for wt, bt, dst in ((wq, qb, qT), (wk, kb, kT)):
                        for m in range(4):
                            ps = ps_pool.tile([128, 512], F32, name="ps", tag="ps")
                            for kc in range(4):
                                nc.tensor.matmul(
                                    ps[:, :512],
                                    wt[kc][:, m * 128:(m + 1) * 128],
                                    xT[kc][:, sl],
                                    start=(kc == 0),
                                    stop=(kc == 3 and not with_qkbias))
                            if with_qkbias:
                                nc.tensor.matmul(
                                    ps[:, :512],
                                    bt[0:1, m * 128:(m + 1) * 128],
                                    ones_r[0:1, :512],
                                    start=False, stop=True)
                            nc.vector.tensor_copy(dst[m][:, sl], ps[:, :512])
                    for sch in range(nch * 4, (nch + 1) * 4):
                        ps = ps_pool.tile([128, 512], F32, name="ps", tag="ps")
                        for kc in range(4):
                            nc.tensor.matmul(
                                ps[:, :512],
                                xT[kc][:, sch * 128:(sch + 1) * 128],
                                wv[kc][:, :512],
                                start=(kc == 0), stop=(kc == 3))
                        nc.vector.tensor_copy(v_sb[sch][:], ps[:, :512])

                # ---- phase 2: attention, software-pipelined over batches ----
                def stage_front(b):
                    """logits -> exp -> bias-mul; returns E tiles for batch b.

                    pl tile (2 banks) holds heads n0=2i (cols 0:256) and n0+1
                    (cols 512:768); row-packed concurrent matmuls must hit
                    distinct PSUM banks. exp covers the full [128, 1024] tile
                    (incl. stale cols) so ACT per-instr overhead amortizes;
                    valid head n lands at e0 cols 512n:512n+256.
                    """
                    ssl = slice(b * S, (b + 1) * S)
                    E = []
                    for tch in range(2):
                        e = epool.tile([128, NH * S], BF16, name="e", tag="e", bufs=4)
                        e0 = (epool.tile([128, NH * S], BF16, name="e0",
                                         tag="e0", bufs=2)
                              if bias_mode == 'mult' else e)
                        tsl = slice(b * S + tch * 128, b * S + tch * 128 + 128)
                        for hg in range(4):
                            for hp in range(2):
                                pl = pl_pool.tile([128, 1024], F32, name="pl", tag="pl")
                                for hi in range(2):
                                    hl = 2 * hp + hi
                                    nc.tensor.matmul(
                                        pl[:, hi * 512:hi * 512 + 256],
                                        kT[hg][32 * hl:32 * hl + 32, tsl],
                                        qT[hg][32 * hl:32 * hl + 32, ssl],
                                        start=True,
                                        stop=(bias_mode != 'pe'),
                                        tile_position=(32 * hl, 0))
                                if bias_mode == 'pe':
                                    # accumulate relative bias: col-packed
                                    # identity-block matmuls on the diagonal
                                    for hi in range(2):
                                        n = 4 * hg + 2 * hp + hi
                                        for r in range(4):
                                            nc.tensor.matmul(
                                                pl[32 * r:32 * r + 32,
                                                   hi * 512:hi * 512 + 256],
                                                ident[32 * r:32 * r + 32, 0:32],
                                                biasT_v[32 * r:32 * r + 32,
                                                        tch, n, :],
                                                start=False, stop=(r == 3),
                                                tile_position=(32 * r, 32 * r),
                                                skip_group_check=True)
                                pl_v = pl.rearrange(
                                    "p (h x) -> p h x", h=2)[:, :, :256]
                                n0 = 4 * hg + 2 * hp
                                e_v = e0[:, n0 * 256:(n0 + 2) * 256].rearrange(
                                    "p (h x) -> p h x", h=2)
                                nc.scalar.activation(e_v, pl_v, AF.Exp)
                        if bias_mode == 'mult':
                            ph = pool_heads
                            pc = ph * 256
                            if ph > 0:
                                nc.gpsimd.tensor_tensor(
                                    e[:, 0:pc], e0[:, 0:pc],
                                    expb[tch][:, 0:pc], OP.mult)
                            if ph < NH:
                                nc.vector.tensor_tensor(
                                    e[:, pc:NH * S], e0[:, pc:NH * S],
                                    expb[tch][:, pc:NH * S], OP.mult)
                        E.append(e)
                    return E

                def stage_back1(b, E):
                    """sums -> recip -> AV -> normalize; returns ot tile."""
                    if not do('sums'):
                        return None
                    psum_s = pl_pool.tile([128, 1024], F32, name="pls", tag="pl")
                    for hg in range(4):
                        for j in range(4):
                            n = 4 * hg + j
                            for tch in range(2):
                                nc.tensor.matmul(
                                    psum_s[32 * j:32 * j + 32,
                                           hg * 256:(hg + 1) * 256],
                                    ones_c[:, :32],
                                    E[tch][:, n * 256:(n + 1) * 256],
                                    start=(tch == 0), stop=(tch == 1),
                                    tile_position=(0, 32 * j))
                    r = rpool.tile([128, 1024], F32, name="r", tag="r")
                    nc.vector.reciprocal_approx_fast(r[:], psum_s[:])
                    if not do('av'):
                        return None
                    pa = pa_pool.tile([128, 1024], F32, name="pa",
                                      tag="pl" if pa_in_pl else "pa")
                    for hg in range(4):
                        for j in range(4):
                            n = 4 * hg + j
                            for tch in range(2):
                                nc.tensor.matmul(
                                    pa[32 * j:32 * j + 32,
                                       hg * 256:(hg + 1) * 256],
                                    v_sb[2 * b + tch][:, n * 32:(n + 1) * 32],
                                    E[tch][:, n * 256:(n + 1) * 256],
                                    start=(tch == 0), stop=(tch == 1),
                                    tile_position=(0, 32 * j))
                    ot = otpool.tile([128, 1024], BF16, name="ot", tag="ot")
                    nc.vector.tensor_tensor(ot[:], pa[:], r[:], OP.mult)
                    return ot

                def stage_back2(b, ot):
                    """outproj -> SBUF copy -> DMA out for batch b."""
                    if ot is None or not do('out'):
                        return
                    fs = fpool.tile([128, 1024], BF16, name="f", tag="f")
                    if big_ps:
                        po = ps_pool.tile([128, 1024], F32, name="po", tag="ps")
                        for sch in range(2):
                            for hg in range(4):
                                nc.tensor.matmul(
                                    po[:, sch * 512:(sch + 1) * 512],
                                    ot[:, hg * 256 + sch * 128:
                                       hg * 256 + (sch + 1) * 128],
                                    wo[hg][:, :512],
                                    start=(hg == 0), stop=(hg == 3))
                        nc.vector.tensor_copy(fs[:], po[:])
                    else:
                        for sch in range(2):
                            po = ps_pool.tile([128, 512], F32, name="po", tag="ps")
                            for hg in range(4):
                                nc.tensor.matmul(
                                    po[:, 0:512],
                                    ot[:, hg * 256 + sch * 128:
                                       hg * 256 + (sch + 1) * 128],
                                    wo[hg][:, :512],
                                    start=(hg == 0), stop=(hg == 3))
                            nc.vector.tensor_copy(
                                fs[:, sch * 512:(sch + 1) * 512], po[:, 0:512])
                    dst = out_d[b * S:(b + 1) * S, :].rearrange(
                        "(c p) w -> p c w", p=128)
                    nc.sync.dma_start(dst, fs.rearrange("p (c w) -> p c w", c=2))

                emit_qkv_chunk(0)
                if do('attn'):
                    prev = None   # (b, E) awaiting back1
                    prev2 = None  # (b, ot) awaiting back2
                    for b in range(BPC):
                        E = stage_front(b)
                        if b % 2 == 0 and b // 2 + 1 < 4:
                            emit_qkv_chunk(b // 2 + 1)
                        if prev is not None:
                            ot_prev = stage_back1(prev[0], prev[1])
                            if prev2 is not None:
                                stage_back2(prev2[0], prev2[1])
                            prev2 = (prev[0], ot_prev)
                        prev = (b, E)
                    ot_last = stage_back1(prev[0], prev[1])
                    if prev2 is not None:
                        stage_back2(prev2[0], prev2[1])
                    stage_back2(prev[0], ot_last)
                else:
                    for nch in range(1, 4):
                        emit_qkv_chunk(nch)

    nc.compile()
    return nc


def _bias_tables(rel_emb):
    """expb[tch, t_local, n*256+s] = exp(bias[n, s, t]) with t = tch*128+t_local."""
    idx = np.arange(H)
    rel = idx[None, :] - idx[:, None] + (H - 1)          # [a, b] -> b - a + 15
    # bias[n, s, t] = rel_emb[n, th-sh+15, tw-sw+15]; biasT[n, t, s] = bias[n, s, t]
    rh = rel[:, :]                                        # [sh, th]
    biasT = rel_emb[:, rh.T[:, None, :, None], rel.T[None, :, None, :]]
    # biasT[n, th, tw, sh, sw] = rel_emb[n, th-sh+15, tw-sw+15]
    biasT = biasT.reshape(NH, S, S)                       # [n, t, s]
    bt = np.ascontiguousarray(np.transpose(biasT, (1, 0, 2)))   # [t, n, s]
    bt = bt.reshape(2, 128, NH * S).transpose(1, 0, 2).reshape(128, 2 * NH * S)
    return np.ascontiguousarray(bt).astype(ml_dtypes.bfloat16)


_CACHE = {}


def _get_program(key=1):
    if isinstance(key, tuple):
        reps, with_qkbias = key
    else:
        reps, with_qkbias = key, False
    k = (reps, with_qkbias)
    if k not in _CACHE:
        # fp8 path has no q/k-bias support; fall back to bf16 when present
        _CACHE[k] = build_program(reps, with_qkbias=with_qkbias,
                                  use_fp8=not with_qkbias)
    return _CACHE[k]


def make_in_maps(use_fp8=True, bias_mode='mult', **inputs):
    x = np.asarray(inputs["x"], np.float32)
    q_w = np.asarray(inputs["q_w"], np.float32).reshape(C, NH * D)
    k_w = np.asarray(inputs["k_w"], np.float32).reshape(C, NH * D)
    v_w = np.asarray(inputs["v_w"], np.float32).reshape(C, NH * D)
    o_w = np.asarray(inputs["o_w"], np.float32).reshape(NH * D, C)
    q_b = np.asarray(inputs["q_b"], np.float32).reshape(NH * D)
    k_b = np.asarray(inputs["k_b"], np.float32).reshape(NH * D)
    rel_emb = np.asarray(inputs["rel_emb"], np.float32)

    bf = ml_dtypes.bfloat16
    f8 = ml_dtypes.float8_e4m3
    wq_s = (q_w * SCALE).reshape(4, 128, 512)
    wk_s = k_w.reshape(4, 128, 512)
    wv_s = v_w.reshape(4, 128, 512)
    wo_s = o_w.reshape(4, 128, 512)
    biasT = _bias_tables(rel_emb)
    ident = np.zeros((128, 32), np.float32)
    ident[np.arange(128), np.arange(128) % 32] = 1.0
    ident = ident.astype(bf)
    qb = (q_b * SCALE).reshape(1, 512).astype(bf)
    kb = k_b.reshape(1, 512).astype(bf)
    ones_r = np.ones((1, 512), bf)
    ones_c = np.ones((128, 32), bf)
    base = dict(qb=qb, kb=kb, ones_r=ones_r, ones_c=ones_c)
    if bias_mode == 'pe':
        base.update(biasT=biasT, ident=ident)
    else:
        base["expb"] = np.exp(
            biasT.astype(np.float32)).astype(ml_dtypes.bfloat16)
    if use_fp8:
        # w8 layout per proj: [p, kc, n] with c = kc*128 + p, scaled by
        # FP8_WSCALE into fp8's normal range (copy rescales by 1/FP8_WSCALE)
        w8 = np.concatenate(
            [np.ascontiguousarray(w.transpose(1, 0, 2) * FP8_WSCALE)
             .reshape(128, 2048) for w in (wq_s, wk_s)],
            axis=1).astype(f8)
        wvo_t = np.ascontiguousarray(np.concatenate(
            [wv_s[i] for i in range(4)] + [wo_s[i] for i in range(4)],
            axis=1)).astype(bf)
        base.update(w8=w8, wvo_t=wvo_t)
    else:
        blocks = []
        for i in range(4):
            blocks += [wq_s[i], wk_s[i], wv_s[i]]
        blocks += [wo_s[i] for i in range(4)]
        base["wqkvo"] = np.ascontiguousarray(
            np.concatenate(blocks, axis=1)).astype(bf)

    in_maps = []
    for ci in range(NCORES):
        xc = x[ci * BPC:(ci + 1) * BPC].reshape(TOK, C)
        xT = np.ascontiguousarray(
            xc.T.reshape(4, 128, TOK).transpose(1, 0, 2).reshape(128, 4 * TOK))
        m = dict(base)
        m["xT"] = xT.astype(bf)
        if use_fp8:
            m["x8"] = xT.astype(f8)
        in_maps.append(m)
    return in_maps


def kernel(**inputs):
    q_b = np.asarray(inputs["q_b"], np.float32).reshape(NH * D)
    k_b = np.asarray(inputs["k_b"], np.float32).reshape(NH * D)
    v_b = np.asarray(inputs["v_b"], np.float32).reshape(NH * D)
    o_b = np.asarray(inputs["o_b"], np.float32).reshape(C)
    o_w = np.asarray(inputs["o_w"], np.float32).reshape(NH * D, C)
    with_qkbias = bool(np.any(q_b) or np.any(k_b))
    nc = _get_program((1, with_qkbias))
    in_maps = make_in_maps(use_fp8=not with_qkbias, **inputs)
    res = run_bass_kernel_spmd(nc, in_maps, core_ids=list(range(NCORES)))
    outs = [res.results[ci]["out"].astype(np.float32).reshape(BPC, S, C)
            for ci in range(NCORES)]
    out = np.concatenate(outs, axis=0)
    # v_b rides through attention as a constant (rows of attn sum to 1); o_b is affine
    const = (v_b @ o_w) + o_b
    if np.any(const):
        out = out + const[None, None, :]
    return out



# revision 13
# speedup vs baseline: 3.2430x; 3.2430x over previous
"""Trainium2 Bass kernel for 2D-relative-bias multi-head attention.

Shapes (hardcoded): x [64, 16, 16, 512], 16 heads x 32 dim, S = 256.
Sharding: data-parallel over batch, 8 batches per core on 8 cores.

Per-core device pipeline (all matmuls bf16, fp32 PSUM accumulation):
  qT/kT = W^T @ x^T            [nd, tok]   (PE, K=c)
  v     = x @ Wv               [tok, nd]   (PE)
  logitsT[t,s] 4 heads/tile    (PE, K=32, 4-way row packing -> concurrent)
  E0 = exp(logitsT)            (ACT, PSUM->SBUF bf16, [128,1024] instrs)
  E  = E0 * exp(biasT)         (DVE + Pool split, bias table from host)
  sums = 1^T E (replicated)    (PE, 4-head col-packed, all-ones lhsT)
  out_unT = V^T E              (PE, 4-head col-packed)
  R = 1/sums                   (DVE reciprocal_approx_fast)
  outT = out_unT * R           (DVE)
  final = outT^T @ Wo + o_b    (PE) -> bf16 -> DMA (host casts to f32)
"""

import numpy as np
import ml_dtypes

try:
    import concourse.bass as bass
except ImportError:  # pragma: no cover
    import sys

    sys.path.insert(0, "/opt/trn_rl_repo")
    import concourse.bass as bass
from concourse import bacc

import concourse.mybir as mybir
import concourse.tile as tile
from concourse.bass_utils import run_bass_kernel_spmd

BF16 = mybir.dt.bfloat16
F32 = mybir.dt.float32
FP8 = mybir.dt.float8e4
FP8_WSCALE = 64.0
AF = mybir.ActivationFunctionType
OP = mybir.AluOpType

B, H, W, C = 64, 16, 16, 512
NH, D = 16, 32
S = H * W            # 256
NCORES = 8
BPC = B // NCORES    # 8 batches per core
TOK = BPC * S        # 2048 tokens per core
SCALE = D ** -0.5

SCHRAUD_A = 128.0 / np.log(2.0)          # bf16-bits-per-e-fold
SCHRAUD_B = 127.0 * 128 - 9.3            # magic bias (tuned on output err)


def build_program(reps: int = 1, debug: bool = False,
                  sections=('qkv', 'attn', 'sums', 'av', 'out'),
                  with_qkbias: bool = False, use_fp8='k',
                  bias_mode: str = 'mult', pool_heads: int = 8,
                  pa_in_pl: bool = True, v_act: bool = False,
                  qk_bufs: int = 1, big_ps: bool = False,
                  schraud: int = 0, act_evac: tuple = ()):
    nc = bacc.Bacc()
    xT_d = nc.dram_tensor("xT", [128, 4 * TOK], BF16, kind="ExternalInput")
    if use_fp8 == 'k':
        x8_d = nc.dram_tensor("x8", [128, 4 * TOK], FP8, kind="ExternalInput")
        w8_d = nc.dram_tensor("w8", [128, 4 * 512], FP8, kind="ExternalInput")
        wqkvo_d = nc.dram_tensor("wqvo_t", [128, 12 * 512], BF16, kind="ExternalInput")
    elif use_fp8:
        x8_d = nc.dram_tensor("x8", [128, 4 * TOK], FP8, kind="ExternalInput")
        w8_d = nc.dram_tensor("w8", [128, 8 * 512], FP8, kind="ExternalInput")
        wqkvo_d = nc.dram_tensor("wvo_t", [128, 8 * 512], BF16, kind="ExternalInput")
    else:
        wqkvo_d = nc.dram_tensor("wqkvo", [128, 16 * 512], BF16, kind="ExternalInput")
    if bias_mode == 'pe':
        biasT_d = nc.dram_tensor("biasT", [128, 2 * NH * S], BF16, kind="ExternalInput")
        ident_d = nc.dram_tensor("ident", [128, 32], BF16, kind="ExternalInput")
    else:
        expb_d = nc.dram_tensor("expb", [128, 2 * NH * S], BF16, kind="ExternalInput")
    qb_d = nc.dram_tensor("qb", [1, 512], BF16, kind="ExternalInput")
    kb_d = nc.dram_tensor("kb", [1, 512], BF16, kind="ExternalInput")
    ones_r_d = nc.dram_tensor("ones_r", [1, 512], BF16, kind="ExternalInput")
    ones_c_d = nc.dram_tensor("ones_c", [128, 32], BF16, kind="ExternalInput")
    out_d = nc.dram_tensor("out", [TOK, 512], BF16, kind="ExternalOutput")

    with tile.TileContext(nc) as tc:
        import contextlib

        with contextlib.ExitStack() as ctx:
            wpool = ctx.enter_context(tc.tile_pool(name="wpool", bufs=1))
            xpool = ctx.enter_context(tc.tile_pool(name="xpool", bufs=1))
            qkpool = ctx.enter_context(tc.tile_pool(name="qkpool", bufs=1))
            epool = ctx.enter_context(tc.tile_pool(name="epool", bufs=3))
            rpool = ctx.enter_context(tc.tile_pool(
                name="rpool", bufs=1 if qk_bufs > 1 else 2))
            otpool = ctx.enter_context(tc.tile_pool(
                name="otpool", bufs=2 if qk_bufs > 1 else 4))
            fpool = ctx.enter_context(tc.tile_pool(
                name="fpool", bufs=1 if qk_bufs > 1 else 2))
            pl_pool = ctx.enter_context(
                tc.tile_pool(name="pl",
                             bufs=2 if big_ps else (3 if pa_in_pl else 2),
                             space="PSUM"))
            pa_pool = (pl_pool if pa_in_pl else ctx.enter_context(
                tc.tile_pool(name="pa", bufs=1, space="PSUM")))
            ps_pool = ctx.enter_context(
                tc.tile_pool(name="ps", bufs=2, space="PSUM"))

            # ---- persistent constants ----
            if use_fp8 == 'k':
                wall = wpool.tile([128, 12 * 512], BF16, name="wall", tag="wall")
                nc.sync.dma_start(wall[:], wqkvo_d[:])
                wq = [wall[:, i * 512:(i + 1) * 512] for i in range(4)]
                wv = [wall[:, (4 + i) * 512:(5 + i) * 512] for i in range(4)]
                wo = [wall[:, (8 + i) * 512:(9 + i) * 512] for i in range(4)]
                x8_all = wpool.tile([128, 4 * TOK], FP8, name="x8", tag="x8")
                nc.sync.dma_start(x8_all[:], x8_d[:])
                x8 = x8_all.rearrange("p (k t) -> p k t", k=4)
                w8_all = wpool.tile([128, 4 * 512], FP8, name="w8", tag="w8")
                nc.sync.dma_start(w8_all[:], w8_d[:])
                w8k = w8_all.rearrange("p (k n) -> p k n", k=4)
            elif use_fp8:
                wall = wpool.tile([128, 8 * 512], BF16, name="wall", tag="wall")
                nc.sync.dma_start(wall[:], wqkvo_d[:])
                wv = [wall[:, i * 512:(i + 1) * 512] for i in range(4)]
                wo = [wall[:, (4 + i) * 512:(5 + i) * 512] for i in range(4)]
                x8_all = wpool.tile([128, 4 * TOK], FP8, name="x8", tag="x8")
                nc.sync.dma_start(x8_all[:], x8_d[:])
                x8 = x8_all.rearrange("p (k t) -> p k t", k=4)
                w8_all = wpool.tile([128, 8 * 512], FP8, name="w8", tag="w8")
                nc.sync.dma_start(w8_all[:], w8_d[:])
                w8 = [w8_all[:, j * 2048:(j + 1) * 2048].rearrange(
                    "p (k n) -> p k n", k=4) for j in range(2)]
            else:
                wall = wpool.tile([128, 16 * 512], BF16, name="wall", tag="wall")
                nc.sync.dma_start(wall[:], wqkvo_d[:])
                wq = [wall[:, (3 * i + 0) * 512:(3 * i + 1) * 512] for i in range(4)]
                wk = [wall[:, (3 * i + 1) * 512:(3 * i + 2) * 512] for i in range(4)]
                wv = [wall[:, (3 * i + 2) * 512:(3 * i + 3) * 512] for i in range(4)]
                wo = [wall[:, (12 + i) * 512:(13 + i) * 512] for i in range(4)]
            if bias_mode == 'pe':
                biasT_all = wpool.tile([128, 2 * NH * S], BF16, name="biasT", tag="biasT")
                nc.sync.dma_start(biasT_all[:], biasT_d[:])
                biasT_v = biasT_all.rearrange("p (t n s) -> p t n s", t=2, n=NH)
                ident = wpool.tile([128, 32], BF16, name="ident", tag="ident")
                nc.sync.dma_start(ident[:], ident_d[:])
            else:
                expb_all = wpool.tile([128, 2 * NH * S], BF16, name="expb", tag="expb")
                nc.sync.dma_start(expb_all[:], expb_d[:])
                expb = [expb_all[:, t * NH * S:(t + 1) * NH * S] for t in range(2)]
            qb = wpool.tile([1, 512], BF16, name="qb", tag="qb")
            kb = wpool.tile([1, 512], BF16, name="kb", tag="kb")
            ones_r = wpool.tile([1, 512], BF16, name="ones_r", tag="ones_r")
            ones_c = wpool.tile([128, 32], BF16, name="ones_c", tag="ones_c")
            nc.sync.dma_start(qb[:], qb_d[:])
            nc.sync.dma_start(kb[:], kb_d[:])
            nc.sync.dma_start(ones_r[:], ones_r_d[:])
            nc.sync.dma_start(ones_c[:], ones_c_d[:])
            xT_all = xpool.tile([128, 4 * TOK], BF16, name="xT", tag="xT")
            nc.sync.dma_start(xT_all[:], xT_d[:])
            xT = [xT_all[:, i * TOK:(i + 1) * TOK] for i in range(4)]

            for _rep in range(reps):
                do = lambda s: s in sections
                # ---- phase 1: QKV projection chunk emitter (interleaved) ----
                if big_ps:
                    qTall = qkpool.tile([128, 4 * TOK], BF16, name="qTall",
                                        tag="qTall", bufs=1)
                    kTall = qkpool.tile([128, 4 * TOK], BF16, name="kTall",
                                        tag="kTall", bufs=1)
                    vall = qkpool.tile([128, 16 * 512], BF16, name="vall",
                                       tag="vall", bufs=1)
                    qT = [qTall[:, m * TOK:(m + 1) * TOK] for m in range(4)]
                    kT = [kTall[:, m * TOK:(m + 1) * TOK] for m in range(4)]
                    v_sb = [vall[:, s * 512:(s + 1) * 512]
                            for s in range(TOK // 128)]
                else:
                    qT = [qkpool.tile([128, TOK], BF16, name=f"qT{m}",
                                      tag=f"qT{m}", bufs=qk_bufs) for m in range(4)]
                    kT = [qkpool.tile([128, TOK], BF16, name=f"kT{m}",
                                      tag=f"kT{m}", bufs=qk_bufs) for m in range(4)]
                    v_sb = [qkpool.tile([128, 512], BF16, name=f"v{s}",
                                        tag=f"v{s}", bufs=1)
                            for s in range(TOK // 128)]

                DR = mybir.MatmulPerfMode.DoubleRow
                INV_WS = 1.0 / FP8_WSCALE

                def emit_qkv_chunk(nch):
                    """q,k projections for token chunk nch (512 tokens) + v for its 4 s-chunks."""
                    sl = slice(nch * 512, (nch + 1) * 512)
                    if use_fp8 == 'k':
                        # q in bf16 (precision), k in fp8-DR (speed)
                        if big_ps:
                            for mp in range(2):
                                ps = ps_pool.tile([128, 1024], F32, name="ps", tag="ps")
                                for mi in range(2):
                                    m = 2 * mp + mi
                                    for kc in range(4):
                                        nc.tensor.matmul(
                                            ps[:, mi * 512:(mi + 1) * 512],
                                            wq[kc][:, m * 128:(m + 1) * 128],
                                            xT[kc][:, sl],
                                            start=(kc == 0), stop=(kc == 3))
                                dst = qTall.rearrange(
                                    "p (m t) -> p m t", m=4)[:, 2 * mp:2 * mp + 2, sl]
                                nc.vector.tensor_copy(
                                    dst, ps.rearrange("p (m t) -> p m t", m=2))
                            for mp in range(2):
                                ps = ps_pool.tile([128, 1024], F32, name="ps", tag="ps")
                                for mi in range(2):
                                    m = 2 * mp + mi
                                    for kp in range(2):
                                        nc.tensor.matmul(
                                            ps[:, mi * 512:(mi + 1) * 512],
                                            w8k[:, 2 * kp:2 * kp + 2,
                                                m * 128:(m + 1) * 128],
                                            x8[:, 2 * kp:2 * kp + 2, sl],
                                            start=(kp == 0), stop=(kp == 1),
                                            perf_mode=DR)
                                dst = kTall.rearrange(
                                    "p (m t) -> p m t", m=4)[:, 2 * mp:2 * mp + 2, sl]
                                nc.vector.tensor_scalar_mul(
                                    dst, ps.rearrange("p (m t) -> p m t", m=2),
                                    INV_WS)
                            for sp in range(2):
                                ps = ps_pool.tile([128, 1024], F32, name="ps", tag="ps")
                                for si in range(2):
                                    sch = nch * 4 + 2 * sp + si
                                    for kc in range(4):
                                        nc.tensor.matmul(
                                            ps[:, si * 512:(si + 1) * 512],
                                            xT[kc][:, sch * 128:(sch + 1) * 128],
                                            wv[kc][:, :512],
                                            start=(kc == 0), stop=(kc == 3))
                                sch0 = nch * 4 + 2 * sp
                                nc.vector.tensor_copy(
                                    vall[:, sch0 * 512:(sch0 + 2) * 512], ps[:])
                        else:
                            for m in range(4):
                                ps = ps_pool.tile([128, 512], F32, name="ps", tag="ps")
                                for kc in range(4):
                                    nc.tensor.matmul(
                                        ps[:, :512],
                                        wq[kc][:, m * 128:(m + 1) * 128],
                                        xT[kc][:, sl],
                                        start=(kc == 0), stop=(kc == 3))
                                if 'q' in act_evac:
                                    nc.scalar.copy(qT[m][:, sl], ps[:, :512])
                                else:
                                    nc.vector.tensor_copy(qT[m][:, sl], ps[:, :512])
                            for m in range(4):
                                ps = ps_pool.tile([128, 512], F32, name="ps", tag="ps")
                                for kp in range(2):
                                    nc.tensor.matmul(
                                        ps[:, :512],
                                        w8k[:, 2 * kp:2 * kp + 2,
                                            m * 128:(m + 1) * 128],
                                        x8[:, 2 * kp:2 * kp + 2, sl],
                                        start=(kp == 0), stop=(kp == 1),
                                        perf_mode=DR)
                                if 'k' in act_evac:
                                    nc.scalar.mul(kT[m][:, sl], ps[:, :512], INV_WS)
                                else:
                                    nc.vector.tensor_scalar_mul(
                                        kT[m][:, sl], ps[:, :512], INV_WS)
                            for sch in range(nch * 4, (nch + 1) * 4):
                                ps = ps_pool.tile([128, 512], F32, name="ps", tag="ps")
                                for kc in range(4):
                                    nc.tensor.matmul(
                                        ps[:, :512],
                                        xT[kc][:, sch * 128:(sch + 1) * 128],
                                        wv[kc][:, :512],
                                        start=(kc == 0), stop=(kc == 3))
                                if 'v' in act_evac:
                                    nc.scalar.copy(v_sb[sch][:], ps[:, :512])
                                else:
                                    nc.vector.tensor_copy(v_sb[sch][:], ps[:, :512])
                        return
                    if use_fp8 and big_ps:
                        for j, dall in ((0, qTall), (1, kTall)):
                            for mp in range(2):
                                ps = ps_pool.tile([128, 1024], F32, name="ps", tag="ps")
                                for mi in range(2):
                                    m = 2 * mp + mi
                                    for kp in range(2):
                                        nc.tensor.matmul(
                                            ps[:, mi * 512:(mi + 1) * 512],
                                            w8[j][:, 2 * kp:2 * kp + 2,
                                                  m * 128:(m + 1) * 128],
                                            x8[:, 2 * kp:2 * kp + 2, sl],
                                            start=(kp == 0), stop=(kp == 1),
                                            perf_mode=DR)
                                dst = dall.rearrange(
                                    "p (m t) -> p m t", m=4
                                )[:, 2 * mp:2 * mp + 2, sl]
                                nc.vector.tensor_scalar_mul(
                                    dst, ps.rearrange("p (m t) -> p m t", m=2),
                                    INV_WS)
                        for sp in range(2):
                            ps = ps_pool.tile([128, 1024], F32, name="ps", tag="ps")
                            for si in range(2):
                                sch = nch * 4 + 2 * sp + si
                                for kc in range(4):
                                    nc.tensor.matmul(
                                        ps[:, si * 512:(si + 1) * 512],
                                        xT[kc][:, sch * 128:(sch + 1) * 128],
                                        wv[kc][:, :512],
                                        start=(kc == 0), stop=(kc == 3))
                            sch0 = nch * 4 + 2 * sp
                            nc.vector.tensor_copy(
                                vall[:, sch0 * 512:(sch0 + 2) * 512], ps[:])
                        return
                    if use_fp8:
                        for j, dst in ((0, qT), (1, kT)):
                            for m in range(4):
                                ps = ps_pool.tile([128, 512], F32, name="ps", tag="ps")
                                for kp in range(2):
                                    nc.tensor.matmul(
                                        ps[:, :512],
                                        w8[j][:, 2 * kp:2 * kp + 2,
                                              m * 128:(m + 1) * 128],
                                        x8[:, 2 * kp:2 * kp + 2, sl],
                                        start=(kp == 0), stop=(kp == 1),
                                        perf_mode=DR)
                                nc.vector.tensor_scalar_mul(
                                    dst[m][:, sl], ps[:, :512], INV_WS)
                        for sch in range(nch * 4, (nch + 1) * 4):
                            ps = ps_pool.tile([128, 512], F32, name="ps", tag="ps")
                            for kc in range(4):
                                nc.tensor.matmul(
                                    ps[:, :512],
                                    xT[kc][:, sch * 128:(sch + 1) * 128],
                                    wv[kc][:, :512],
                                    start=(kc == 0), stop=(kc == 3))
                            if v_act:
                                nc.scalar.activation(
                                    v_sb[sch][:], ps[:, :512], AF.Copy)
                            else:
                                nc.vector.tensor_copy(v_sb[sch][:], ps[:, :512])
                        return
                    for wt, bt, dst in ((wq, qb, qT), (wk, kb, kT)):
                        for m in range(4):
                            ps = ps_pool.tile([128, 512], F32, name="ps", tag="ps")
                            for kc in range(4):
                                nc.tensor.matmul(
                                    ps[:, :512],
                                    wt[kc][:, m * 128:(m + 1) * 128],
                                    xT[kc][:, sl],
                                    start=(kc == 0),
                                    stop=(kc == 3 and not with_qkbias))
                            if with_qkbias:
                                nc.tensor.matmul(
                                    ps[:, :512],
                                    bt[0:1, m * 128:(m + 1) * 128],
                                    ones_r[0:1, :512],
                                    start=False, stop=True)
                            nc.vector.tensor_copy(dst[m][:, sl], ps[:, :512])
                    for sch in range(nch * 4, (nch + 1) * 4):
                        ps = ps_pool.tile([128, 512], F32, name="ps", tag="ps")
                        for kc in range(4):
                            nc.tensor.matmul(
                                ps[:, :512],
                                xT[kc][:, sch * 128:(sch + 1) * 128],
                                wv[kc][:, :512],
                                start=(kc == 0), stop=(kc == 3))
                        nc.vector.tensor_copy(v_sb[sch][:], ps[:, :512])

                # ---- phase 2: attention, software-pipelined over batches ----
                def stage_front(b):
                    """logits -> exp -> bias-mul; returns E tiles for batch b.

                    pl tile (2 banks) holds heads n0=2i (cols 0:256) and n0+1
                    (cols 512:768); row-packed concurrent matmuls must hit
                    distinct PSUM banks. exp covers the full [128, 1024] tile
                    (incl. stale cols) so ACT per-instr overhead amortizes;
                    valid head n lands at e0 cols 512n:512n+256.
                    """
                    ssl = slice(b * S, (b + 1) * S)
                    E = []
                    for tch in range(2):
                        e = epool.tile([128, NH * S], BF16, name="e", tag="e", bufs=4)
                        e0 = (epool.tile([128, NH * S], BF16, name="e0",
                                         tag="e0", bufs=2)
                              if bias_mode == 'mult' else e)
                        tsl = slice(b * S + tch * 128, b * S + tch * 128 + 128)
                        for hg in range(4):
                            for hp in range(2):
                                pl = pl_pool.tile([128, 1024], F32, name="pl", tag="pl")
                                for hi in range(2):
                                    hl = 2 * hp + hi
                                    nc.tensor.matmul(
                                        pl[:, hi * 512:hi * 512 + 256],
                                        kT[hg][32 * hl:32 * hl + 32, tsl],
                                        qT[hg][32 * hl:32 * hl + 32, ssl],
                                        start=True,
                                        stop=(bias_mode != 'pe'),
                                        tile_position=(32 * hl, 0))
                                if bias_mode == 'pe':
                                    # accumulate relative bias: col-packed
                                    # identity-block matmuls on the diagonal
                                    for hi in range(2):
                                        n = 4 * hg + 2 * hp + hi
                                        for r in range(4):
                                            nc.tensor.matmul(
                                                pl[32 * r:32 * r + 32,
                                                   hi * 512:hi * 512 + 256],
                                                ident[32 * r:32 * r + 32, 0:32],
                                                biasT_v[32 * r:32 * r + 32,
                                                        tch, n, :],
                                                start=False, stop=(r == 3),
                                                tile_position=(32 * r, 32 * r),
                                                skip_group_check=True)
                                pl_v = pl.rearrange(
                                    "p (h x) -> p h x", h=2)[:, :, :256]
                                n0 = 4 * hg + 2 * hp
                                if 2 * hg + hp < schraud:
                                    # Schraudolph bf16 exp on DVE: bits(bf16
                                    # exp(x)) ~= round(x*128/ln2 + B); write
                                    # int16 into e0's bytes, read back as bf16
                                    e_v16 = e0[:, n0 * 256:(n0 + 2) * 256] \
                                        .bitcast(mybir.dt.int16).rearrange(
                                            "p (h x) -> p h x", h=2)
                                    nc.vector.tensor_scalar(
                                        e_v16, pl_v, SCHRAUD_A, SCHRAUD_B,
                                        op0=OP.mult, op1=OP.add)
                                else:
                                    e_v = e0[:, n0 * 256:(n0 + 2) * 256] \
                                        .rearrange("p (h x) -> p h x", h=2)
                                    nc.scalar.activation(e_v, pl_v, AF.Exp)
                        if bias_mode == 'mult':
                            ph = pool_heads
                            pc = ph * 256
                            if ph > 0:
                                nc.gpsimd.tensor_tensor(
                                    e[:, 0:pc], e0[:, 0:pc],
                                    expb[tch][:, 0:pc], OP.mult)
                            if ph < NH:
                                nc.vector.tensor_tensor(
                                    e[:, pc:NH * S], e0[:, pc:NH * S],
                                    expb[tch][:, pc:NH * S], OP.mult)
                        E.append(e)
                    return E

                def stage_back1(b, E):
                    """sums -> recip -> AV -> normalize; returns ot tile."""
                    if not do('sums'):
                        return None
                    psum_s = pl_pool.tile([128, 1024], F32, name="pls", tag="pl")
                    for hg in range(4):
                        for j in range(4):
                            n = 4 * hg + j
                            for tch in range(2):
                                nc.tensor.matmul(
                                    psum_s[32 * j:32 * j + 32,
                                           hg * 256:(hg + 1) * 256],
                                    ones_c[:, :32],
                                    E[tch][:, n * 256:(n + 1) * 256],
                                    start=(tch == 0), stop=(tch == 1),
                                    tile_position=(0, 32 * j))
                    r = rpool.tile([128, 1024], F32, name="r", tag="r")
                    nc.vector.reciprocal_approx_fast(r[:], psum_s[:])
                    if not do('av'):
                        return None
                    pa = pa_pool.tile([128, 1024], F32, name="pa",
                                      tag="pl" if pa_in_pl else "pa")
                    for hg in range(4):
                        for j in range(4):
                            n = 4 * hg + j
                            for tch in range(2):
                                nc.tensor.matmul(
                                    pa[32 * j:32 * j + 32,
                                       hg * 256:(hg + 1) * 256],
                                    v_sb[2 * b + tch][:, n * 32:(n + 1) * 32],
                                    E[tch][:, n * 256:(n + 1) * 256],
                                    start=(tch == 0), stop=(tch == 1),
                                    tile_position=(0, 32 * j))
                    ot = otpool.tile([128, 1024], BF16, name="ot", tag="ot")
                    nc.vector.tensor_tensor(ot[:], pa[:], r[:], OP.mult)
                    return ot

                def stage_back2(b, ot):
                    """outproj -> SBUF copy -> DMA out for batch b."""
                    if ot is None or not do('out'):
                        return
                    fs = fpool.tile([128, 1024], BF16, name="f", tag="f")
                    if big_ps:
                        po = ps_pool.tile([128, 1024], F32, name="po", tag="ps")
                        for sch in range(2):
                            for hg in range(4):
                                nc.tensor.matmul(
                                    po[:, sch * 512:(sch + 1) * 512],
                                    ot[:, hg * 256 + sch * 128:
                                       hg * 256 + (sch + 1) * 128],
                                    wo[hg][:, :512],
                                    start=(hg == 0), stop=(hg == 3))
                        nc.vector.tensor_copy(fs[:], po[:])
                    else:
                        for sch in range(2):
                            po = ps_pool.tile([128, 512], F32, name="po", tag="ps")
                            for hg in range(4):
                                nc.tensor.matmul(
                                    po[:, 0:512],
                                    ot[:, hg * 256 + sch * 128:
                                       hg * 256 + (sch + 1) * 128],
                                    wo[hg][:, :512],
                                    start=(hg == 0), stop=(hg == 3))
                            if 'fs' in act_evac:
                                nc.scalar.copy(
                                    fs[:, sch * 512:(sch + 1) * 512], po[:, 0:512])
                            else:
                                nc.vector.tensor_copy(
                                    fs[:, sch * 512:(sch + 1) * 512], po[:, 0:512])
                    dst = out_d[b * S:(b + 1) * S, :].rearrange(
                        "(c p) w -> p c w", p=128)
                    nc.sync.dma_start(dst, fs.rearrange("p (c w) -> p c w", c=2))

                emit_qkv_chunk(0)
                if do('attn'):
                    prev = None   # (b, E) awaiting back1
                    prev2 = None  # (b, ot) awaiting back2
                    for b in range(BPC):
                        E = stage_front(b)
                        if b % 2 == 0 and b // 2 + 1 < 4:
                            emit_qkv_chunk(b // 2 + 1)
                        if prev is not None:
                            ot_prev = stage_back1(prev[0], prev[1])
                            if prev2 is not None:
                                stage_back2(prev2[0], prev2[1])
                            prev2 = (prev[0], ot_prev)
                        prev = (b, E)
                    ot_last = stage_back1(prev[0], prev[1])
                    if prev2 is not None:
                        stage_back2(prev2[0], prev2[1])
                    stage_back2(prev[0], ot_last)
                else:
                    for nch in range(1, 4):
                        emit_qkv_chunk(nch)

    nc.compile()
    return nc


def _bias_tables(rel_emb):
    """expb[tch, t_local, n*256+s] = exp(bias[n, s, t]) with t = tch*128+t_local."""
    idx = np.arange(H)
    rel = idx[None, :] - idx[:, None] + (H - 1)          # [a, b] -> b - a + 15
    # bias[n, s, t] = rel_emb[n, th-sh+15, tw-sw+15]; biasT[n, t, s] = bias[n, s, t]
    rh = rel[:, :]                                        # [sh, th]
    biasT = rel_emb[:, rh.T[:, None, :, None], rel.T[None, :, None, :]]
    # biasT[n, th, tw, sh, sw] = rel_emb[n, th-sh+15, tw-sw+15]
    biasT = biasT.reshape(NH, S, S)                       # [n, t, s]
    bt = np.ascontiguousarray(np.transpose(biasT, (1, 0, 2)))   # [t, n, s]
    bt = bt.reshape(2, 128, NH * S).transpose(1, 0, 2).reshape(128, 2 * NH * S)
    return np.ascontiguousarray(bt).astype(ml_dtypes.bfloat16)


_CACHE = {}


def _get_program(key=1):
    if isinstance(key, tuple):
        reps, with_qkbias = key
    else:
        reps, with_qkbias = key, False
    k = (reps, with_qkbias)
    if k not in _CACHE:
        # fp8 path has no q/k-bias support; fall back to bf16 when present
        _CACHE[k] = build_program(reps, with_qkbias=with_qkbias,
                                  use_fp8='k' if not with_qkbias else False)
    return _CACHE[k]


def make_in_maps(use_fp8='k', bias_mode='mult', **inputs):
    x = np.asarray(inputs["x"], np.float32)
    q_w = np.asarray(inputs["q_w"], np.float32).reshape(C, NH * D)
    k_w = np.asarray(inputs["k_w"], np.float32).reshape(C, NH * D)
    v_w = np.asarray(inputs["v_w"], np.float32).reshape(C, NH * D)
    o_w = np.asarray(inputs["o_w"], np.float32).reshape(NH * D, C)
    q_b = np.asarray(inputs["q_b"], np.float32).reshape(NH * D)
    k_b = np.asarray(inputs["k_b"], np.float32).reshape(NH * D)
    rel_emb = np.asarray(inputs["rel_emb"], np.float32)

    bf = ml_dtypes.bfloat16
    f8 = ml_dtypes.float8_e4m3
    wq_s = (q_w * SCALE).reshape(4, 128, 512)
    wk_s = k_w.reshape(4, 128, 512)
    wv_s = v_w.reshape(4, 128, 512)
    wo_s = o_w.reshape(4, 128, 512)
    biasT = _bias_tables(rel_emb)
    ident = np.zeros((128, 32), np.float32)
    ident[np.arange(128), np.arange(128) % 32] = 1.0
    ident = ident.astype(bf)
    qb = (q_b * SCALE).reshape(1, 512).astype(bf)
    kb = k_b.reshape(1, 512).astype(bf)
    ones_r = np.ones((1, 512), bf)
    ones_c = np.ones((128, 32), bf)
    base = dict(qb=qb, kb=kb, ones_r=ones_r, ones_c=ones_c)
    if bias_mode == 'pe':
        base.update(biasT=biasT, ident=ident)
    else:
        base["expb"] = np.exp(
            biasT.astype(np.float32)).astype(ml_dtypes.bfloat16)
    if use_fp8 == 'k':
        w8 = np.ascontiguousarray(
            wk_s.transpose(1, 0, 2) * FP8_WSCALE).reshape(128, 2048).astype(f8)
        wqvo_t = np.ascontiguousarray(np.concatenate(
            [wq_s[i] for i in range(4)] + [wv_s[i] for i in range(4)]
            + [wo_s[i] for i in range(4)], axis=1)).astype(bf)
        base.update(w8=w8, wqvo_t=wqvo_t)
    elif use_fp8:
        # w8 layout per proj: [p, kc, n] with c = kc*128 + p, scaled by
        # FP8_WSCALE into fp8's normal range (copy rescales by 1/FP8_WSCALE)
        w8 = np.concatenate(
            [np.ascontiguousarray(w.transpose(1, 0, 2) * FP8_WSCALE)
             .reshape(128, 2048) for w in (wq_s, wk_s)],
            axis=1).astype(f8)
        wvo_t = np.ascontiguousarray(np.concatenate(
            [wv_s[i] for i in range(4)] + [wo_s[i] for i in range(4)],
            axis=1)).astype(bf)
        base.update(w8=w8, wvo_t=wvo_t)
    else:
        blocks = []
        for i in range(4):
            blocks += [wq_s[i], wk_s[i], wv_s[i]]
        blocks += [wo_s[i] for i in range(4)]
        base["wqkvo"] = np.ascontiguousarray(
            np.concatenate(blocks, axis=1)).astype(bf)

    in_maps = []
    for ci in range(NCORES):
        xc = x[ci * BPC:(ci + 1) * BPC].reshape(TOK, C)
        xT = np.ascontiguousarray(
            xc.T.reshape(4, 128, TOK).transpose(1, 0, 2).reshape(128, 4 * TOK))
        m = dict(base)
        m["xT"] = xT.astype(bf)
        if use_fp8:
            m["x8"] = xT.astype(f8)
        in_maps.append(m)
    return in_maps


def kernel(**inputs):
    q_b = np.asarray(inputs["q_b"], np.float32).reshape(NH * D)
    k_b = np.asarray(inputs["k_b"], np.float32).reshape(NH * D)
    v_b = np.asarray(inputs["v_b"], np.float32).reshape(NH * D)
    o_b = np.asarray(inputs["o_b"], np.float32).reshape(C)
    o_w = np.asarray(inputs["o_w"], np.float32).reshape(NH * D, C)
    with_qkbias = bool(np.any(q_b) or np.any(k_b))
    nc = _get_program((1, with_qkbias))
    in_maps = make_in_maps(use_fp8='k' if not with_qkbias else False, **inputs)
    res = run_bass_kernel_spmd(nc, in_maps, core_ids=list(range(NCORES)))
    outs = [res.results[ci]["out"].astype(np.float32).reshape(BPC, S, C)
            for ci in range(NCORES)]
    out = np.concatenate(outs, axis=0)
    # v_b rides through attention as a constant (rows of attn sum to 1); o_b is affine
    const = (v_b @ o_w) + o_b
    if np.any(const):
        out = out + const[None, None, :]
    return out

